# revision 5
# baseline (speedup 1.0000x reference)
"""Trainium2 Bass kernel for a single-layer RNN (tanh) + final linear. v2.

Problem: B=64, T=512, I=256, H=1024, O=128 (fp32).
    xp = einsum('bti,hi->tbh', x, W_ih) + b_ih + b_hh
    h_t = tanh(xp_t + h_{t-1} @ W_hh.T)         (T sequential steps)
    y   = h_T @ W_lin.T + b_lin
Sharding: data-parallel over batch, 8 cores x 8 rows each.

v2 changes vs baseline:
 1. Half-split recurrence rounds: each k-chunk round f is split into two
    N=128 matmuls (PSUM column halves A/B), so tanh+transpose of half A
    fires after the A-column rounds complete instead of after the whole
    step.  Two tanh->transpose chains per step overlap PE streaming.
 2. Bulk input projection: xp for 16 timesteps is computed in one PSUM
    block [128(t',b), 1024] (W_ih streamed once per 16 steps, amortized
    4x) + bias via ones-matmul, copied to SBUF bf16, and injected into
    each step's PSUM via a K=8 selector matmul (N=128 per half).  This
    removes the per-step proj (2 rounds) + bias (1 round) from the PE.

Layouts (per core):
  PSUM step tile: ps[32j+b, n] = z[b, 256j+n]; half A = cols 0:128
  (hidden 256j+0..127 <-> stationary slices f=0..3), half B = cols
  128:256 (f=4..7).
  S[32j+b, n] = tanh(z); T = 32x32 block transpose of S:
  t_slice(T, f) = T[:, 128*(f//4)+32*(f%4) : +8] is the [128, 8]
  stationary operand for rec round f.
"""

import os
import sys

import ml_dtypes
import numpy as np

BF16 = ml_dtypes.bfloat16

for _p in ("/root/.axon_site", "/root/.axon_site/_ro/trn_rl_repo",
           "/root/.axon_site/_ro/pypackages", "/opt/trn_rl_repo"):
    if os.path.isdir(_p) and _p not in sys.path:
        sys.path.append(_p)

B, I, H, O = 64, 256, 1024, 128
NCORES = 8
B_LOC = B // NCORES  # 8
TBLK = 16            # timesteps per bulk-projection block

_module_cache = {}


def _strip_pe_ticks(nc):
    """Drop the per-matmul PE tick-sem increments that nothing targets.

    Every InstMatmult carries a +1 update on the PE tick semaphore, and the
    hardware drains these at ~34ns each.  Keep only the increments whose tick
    index is the exact target of some wait (plus the last), and remap every
    wait value to its rank among kept ticks.
    """
    import concourse.mybir as mybir

    f = nc.m.functions[0]
    insts = []
    for bb in f.blocks:
        insts.extend(bb.instructions)
    tickers = []
    for i in insts:
        if isinstance(i, mybir.InstMatmult) and i.sync_info:
            for u in i.sync_info.on_update:
                if u.update_mode == "sem-inc":
                    tickers.append((i, u.id))
    if not tickers:
        return
    semid = tickers[0][1]
    assert all(s == semid for _, s in tickers)
    waits = []
    for i in insts:
        si = i.sync_info
        if not si:
            continue
        for w in si.on_wait:
            if w.id == semid:
                assert w.wait_mode == "sem-ge-imm" and w.wait_reg is None
                waits.append(w)
    n = len(tickers)
    keep = {n - 1}
    for w in waits:
        assert 1 <= w.wait_value <= n, (w.wait_value, n)
        keep.add(w.wait_value - 1)
    rank = [0] * n
    c = 0
    for idx in range(n):
        if idx in keep:
            c += 1
        rank[idx] = c
    for w in waits:
        w.wait_value = rank[w.wait_value - 1]
    for idx, (i, _) in enumerate(tickers):
        if idx not in keep:
            i.sync_info.on_update = [
                u for u in i.sync_info.on_update
                if not (u.id == semid and u.update_mode == "sem-inc")]


def _build_module(t_steps, sim=False):
    key = (t_steps, sim)
    if key in _module_cache:
        return _module_cache[key]

    from contextlib import ExitStack

    import concourse.bacc as bacc
    import concourse.mybir as mybir
    import concourse.tile as tile
    from concourse.tile_rust import add_dep_helper

    assert t_steps % TBLK == 0
    nblk = t_steps // TBLK

    f32 = mybir.dt.float32
    bf16 = mybir.dt.bfloat16
    Tanh = mybir.ActivationFunctionType.Tanh

    nc = bacc.Bacc("TRN2", target_bir_lowering=False, debug=False,
                   enable_asserts=False)

    # xTb[p, (blk*2+k)*128 + 8*t' + b] = x[b, 16*blk+t', 128k+p]
    xTb_d = nc.dram_tensor("xTb", [128, 2 * t_steps * B_LOC], bf16,
                           kind="ExternalInput")
    wt_d = nc.dram_tensor("wt", [128, 8 * H], bf16, kind="ExternalInput")
    wih_d = nc.dram_tensor("wih", [128, 2 * H], bf16, kind="ExternalInput")
    wlin_d = nc.dram_tensor("wlin", [128, 8 * O], bf16, kind="ExternalInput")
    bias_d = nc.dram_tensor("bias1", [1, H], bf16, kind="ExternalInput")
    sel_d = nc.dram_tensor("sel", [128, 32], bf16, kind="ExternalInput")
    y_d = nc.dram_tensor("y", [B_LOC, O], f32, kind="ExternalOutput")

    with tile.TileContext(nc) as tc, ExitStack() as ctx:
        wpool = ctx.enter_context(tc.tile_pool(name="weights", bufs=1))
        ppool = ctx.enter_context(tc.tile_pool(name="psum", bufs=3,
                                               space="PSUM"))

        # Small inputs first (block-0 bulk proj gates on them), then the
        # 2 MB wt, then the rest of xTb lazily.
        sel_sb = wpool.tile([128, 32], bf16, name="sel_sb")
        nc.sync.dma_start(out=sel_sb, in_=sel_d.ap())
        bias_sb = wpool.tile([1, H], bf16, name="bias_sb")
        nc.sync.dma_start(out=bias_sb, in_=bias_d.ap())
        wih_sb = wpool.tile([128, 2 * H], bf16, name="wih_sb")
        nc.sync.dma_start(out=wih_sb, in_=wih_d.ap())
        xTb_sb = wpool.tile([128, 2 * t_steps * B_LOC], bf16, name="xTb_sb")
        CHUNK = 2 * TBLK * B_LOC  # one block = 256 cols
        nc.sync.dma_start(out=xTb_sb[:, 0:CHUNK], in_=xTb_d.ap()[:, 0:CHUNK])
        wt_sb = wpool.tile([128, 8 * H], bf16, name="wt_sb")
        nc.sync.dma_start(out=wt_sb, in_=wt_d.ap())
        wlin_sb = wpool.tile([128, 8 * O], bf16, name="wlin_sb")
        nc.sync.dma_start(out=wlin_sb, in_=wlin_d.ap())
        for c in range(CHUNK, 2 * t_steps * B_LOC, 4 * CHUNK):
            ce = min(c + 4 * CHUNK, 2 * t_steps * B_LOC)
            nc.sync.dma_start(out=xTb_sb[:, c:ce], in_=xTb_d.ap()[:, c:ce])
        ones_sb = wpool.tile([1, 32], bf16, name="ones_sb")
        nc.vector.memset(ones_sb, 1.0)

        # xp PSUM block [128, 1024] (q=8t'+b rows, ho cols), one at a time.
        xp_ps = ppool.tile([128, H], f32, name="xp_ps", tag="xp", bufs=1)

        # HAM warmup: back-to-back dummy matmuls so the PE clock reaches
        # 2.4 GHz before the recurrence; scribbles on the xp PSUM block,
        # which the block-0 bias rounds (start=True) overwrite right after.
        warm_sb = wpool.tile([128, 512], bf16, name="warm_sb")
        nc.vector.memset(warm_sb, 0.0)
        for _ in range(12):
            nc.tensor.matmul(xp_ps[:, 0:512], warm_sb[:, 0:128], warm_sb,
                             start=True, stop=True, skip_group_check=True,
                             tile_position=(0, 0))
        # xp SBUF double buffer (bf16)
        xp_sb = [wpool.tile([128, H], bf16, name="xp_sb%d" % i)
                 for i in range(2)]

        post_last = [None]

        def link_post(mm):
            # schedule-order only: keep trailing PE MMs after post ops in
            # Tile's global order so tanh's PE-tick target excludes them.
            if post_last[0] is not None:
                add_dep_helper(post_last[0].ins, mm.ins, sync=False,
                               reason="post before trailing mm")
                post_last[0] = None

        def bulk_round(blk, r):
            """One of 6 bulk-proj rounds for block blk.

            r=0,1: bias start rounds (N=512 halves);
            r=2..5: x @ W_ih^T rounds (k-chunk, N=512 half)."""
            if r < 2:
                nh = r
                for j in range(4):
                    mm = nc.tensor.matmul(
                        xp_ps[32 * j:32 * j + 32, 512 * nh:512 * nh + 512],
                        ones_sb,
                        bias_sb[:, 512 * nh:512 * nh + 512],
                        start=True, stop=False, skip_group_check=True,
                        tile_position=(0, 32 * j))
                    link_post(mm)
            else:
                k, nh = (r - 2) // 2, (r - 2) % 2
                base = (blk * 2 + k) * 128
                for j in range(4):
                    mm = nc.tensor.matmul(
                        xp_ps[32 * j:32 * j + 32, 512 * nh:512 * nh + 512],
                        xTb_sb[:, base + 32 * j:base + 32 * j + 32],
                        wih_sb[:, k * H + 512 * nh:k * H + 512 * nh + 512],
                        start=False, stop=(k == 1), skip_group_check=True,
                        tile_position=(0, 32 * j))
                    link_post(mm)

        def bulk_copy(blk, q):
            """Copy xp PSUM quarter to the SBUF bf16 double buffer.

            On DVE (GPSIMD cannot read PSUM), in quarters on different
            steps: the Scalar engine runs the latency-critical tanhs and a
            fat copy between them disrupts the chain phase."""
            dst = xp_sb[blk % 2]
            nc.vector.tensor_copy(out=dst[:, 256 * q:256 * q + 256],
                                  in_=xp_ps[:, 256 * q:256 * q + 256])

        def t_slice(Th, f):
            # Th = (TA, TB): TA holds slices f=0..3, TB holds f=4..7
            o = 32 * (f % 4)
            return Th[f // 4][:, o:o + 8]

        # Block 0 bulk proj + copies before the loop.
        for r in range(6):
            bulk_round(0, r)
        for q in range(4):
            bulk_copy(0, q)

        def mk_psh():
            # separate tiles per column half so tanh of one half has no
            # tile-granularity dependency on the other half's matmuls
            return [ppool.tile([128, 128], f32, name="psA", tag="psA",
                               bufs=3),
                    ppool.tile([128, 128], f32, name="psB", tag="psB",
                               bufs=3)]

        def warm_mm(psd):
            # keep-warm matmul into the next step's PSUM tile (overwritten
            # by the inject's start=True); keeps the PE pipeline hot
            # through the tanh/transpose wait windows.
            mm = nc.tensor.matmul(
                psd[0:8, :], warm_sb[:, 0:8], warm_sb[:, 0:128],
                start=True, stop=True, skip_group_check=True,
                tile_position=(0, 0))
            link_post(mm)

        def inj(x, psd, t0):
            # quadrant-aligned selector read: contraction over the 32
            # partitions [32g:32g+32] of xp_sb picks out rows 8t'..8t'+8
            # via identity columns 8r..8r+8 (t' = 4g + r).
            gg, rr = (t0 % TBLK) // 4, (t0 % TBLK) % 4
            bb = xp_sb[(t0 // TBLK) % 2]
            for j in range(4):
                mm = nc.tensor.matmul(
                    psd[x][32 * j:32 * j + 8, :],
                    sel_sb[32 * gg:32 * gg + 32, 8 * rr:8 * rr + 8],
                    bb[32 * gg:32 * gg + 32,
                       256 * j + 128 * x:256 * j + 128 * x + 128],
                    start=True, stop=(t0 == 0), skip_group_check=True,
                    tile_position=(32 * gg, 32 * j))
                link_post(mm)

        T_prev = None
        psh = mk_psh()
        inj(1, psh, 0)
        inj(0, psh, 0)
        for t in range(t_steps):
            blk, tl = t // TBLK, t % TBLK

            def rec(f, x, stop, psd=psh, Tp=T_prev):
                for j in range(4):
                    mm = nc.tensor.matmul(
                        psd[x][32 * j:32 * j + 8, :],
                        t_slice(Tp, f),
                        wt_sb[:, H * f + 256 * j + 128 * x:
                              H * f + 256 * j + 128 * x + 128],
                        start=False, stop=stop, skip_group_check=True,
                        tile_position=(0, 32 * j))
                    link_post(mm)

            # B half: old dependency (trB_{t-1}) first, fresh last
            if t > 0:
                for f in (4, 5, 6, 7):
                    rec(f, 1, False)
                for f in (0, 1, 2):
                    rec(f, 1, False)
                rec(3, 1, True)
            # interleave next block's bulk work into this block's steps
            if blk + 1 < nblk:
                if 2 <= tl < 8:
                    bulk_round(blk + 1, tl - 2)
                elif tl >= 8 and tl % 2 == 0:
                    bulk_copy(blk + 1, (tl - 8) // 2)
            SA = wpool.tile([128, 128], bf16, name="SA", tag="SA", bufs=4)
            SB = wpool.tile([128, 128], bf16, name="SB", tag="SB", bufs=4)
            TA = wpool.tile([128, 128], bf16, name="TA", tag="TA", bufs=4)
            TB = wpool.tile([128, 128], bf16, name="TB", tag="TB", bufs=4)
            a = nc.scalar.activation(out=SB, in_=psh[1], func=Tanh)
            post_last[0] = a
            nc.vector.transpose(out=TB, in_=SB)
            # next step's PSUM start + keep-warm fill the trB->B47 gap
            psh_next = mk_psh() if t + 1 < t_steps else None
            if psh_next is not None:
                warm_mm(psh_next[1])
                inj(1, psh_next, t + 1)
            # A half: old dependency (trB_{t-1}) first, fresh last
            if t > 0:
                for f in (4, 5, 6, 7):
                    rec(f, 0, False)
                for f in (0, 1, 2):
                    rec(f, 0, False)
                rec(3, 0, True)
            a = nc.scalar.activation(out=SA, in_=psh[0], func=Tanh)
            post_last[0] = a
            nc.vector.transpose(out=TA, in_=SA)
            if psh_next is not None:
                warm_mm(psh_next[0])
                inj(0, psh_next, t + 1)
            T_prev = (TA, TB)
            psh = psh_next

        psf = ppool.tile([128, 128], f32, name="psf", tag="psA", bufs=3)
        nc.vector.memset(psf, 0.0)
        for f in range(8):
            nc.tensor.matmul(
                psf[0:8, :], t_slice(T_prev, f),
                wlin_sb[:, O * f:O * f + O],
                start=(f == 0), stop=(f == 7), skip_group_check=True,
                tile_position=(0, 0))
        y_sb = wpool.tile([B_LOC, O], f32, name="y_sb", tag="y", bufs=1)
        nc.scalar.copy(out=y_sb, in_=psf[0:B_LOC, :])
        nc.sync.dma_start(out=y_d.ap(), in_=y_sb)

    nc.compile()
    try:
        _strip_pe_ticks(nc)
    except AssertionError:
        pass
    _module_cache[key] = nc
    return nc


def _host_inputs(x, W_ih, W_hh, b_ih, b_hh, W_lin):
    t_steps = x.shape[1]
    nblk = t_steps // TBLK
    wt = np.ascontiguousarray(
        W_hh.T.reshape(4, 8, 32, H).transpose(0, 2, 1, 3).reshape(128, 8 * H)
        .astype(BF16))
    wih = np.ascontiguousarray(
        W_ih.T.reshape(2, 128, H).transpose(1, 0, 2).reshape(128, 2 * H)
        .astype(BF16))
    wlin = np.ascontiguousarray(
        W_lin.T.reshape(4, 8, 32, O).transpose(0, 2, 1, 3).reshape(128, 8 * O)
        .astype(BF16))
    bias1 = np.ascontiguousarray((b_ih + b_hh).reshape(1, H).astype(BF16))
    sel = np.tile(np.eye(32, dtype=BF16), (4, 1))

    in_maps = []
    for core in range(NCORES):
        xc = x[core * B_LOC:(core + 1) * B_LOC]  # [8, T, I]
        # [b, n, t', k, p] -> [p, n, k, t', b]
        xTb = np.ascontiguousarray(
            xc.reshape(B_LOC, nblk, TBLK, 2, 128)
            .transpose(4, 1, 3, 2, 0)
            .reshape(128, 2 * t_steps * B_LOC).astype(BF16))
        in_maps.append({"xTb": xTb, "wt": wt, "wih": wih, "wlin": wlin,
                        "bias1": bias1, "sel": sel})
    return in_maps


def kernel(x, W_ih, W_hh, b_ih, b_hh, W_lin, b_lin, _trace=False):
    x = np.asarray(x, np.float32)
    W_ih = np.asarray(W_ih, np.float32)
    W_hh = np.asarray(W_hh, np.float32)
    b_ih = np.asarray(b_ih, np.float32)
    b_hh = np.asarray(b_hh, np.float32)
    W_lin = np.asarray(W_lin, np.float32)
    b_lin = np.asarray(b_lin, np.float32)

    t_steps = x.shape[1]
    nc = _build_module(t_steps)
    in_maps = _host_inputs(x, W_ih, W_hh, b_ih, b_hh, W_lin)

    from concourse.bass_utils import run_bass_kernel_spmd
    res = run_bass_kernel_spmd(nc, in_maps, core_ids=list(range(NCORES)),
                               trace=_trace)
    y = np.concatenate([res.results[c]["y"] for c in range(NCORES)], axis=0)
    if _trace:
        kernel.last_results = res
    return (y + b_lin[None, :]).astype(np.float32)
